# revision 36
# baseline (speedup 1.0000x reference)
"""AttnDecoderRNN step on 8 Trainium2 NeuronCores.

Sharding: out_W column-parallel (vocab) across 8 cores; tiny attention+GRU
replicated on every core; embedding row gathered on host (4KB of emb_W);
log_softmax normalizer combined on host from per-core sum(exp(logits)).

Device kernel notes:
- 1024-dim stationary vectors live as [128, 8] k-tile columns
  (partition p, col k <-> element 128k+p).
- comb/GRU matmuls run in row form (vector columns stationary, weight
  k-row chunks moving); rows convert back to k-tile columns via K=1
  matmuls against ones[1,1].
- Weight streams use few large DMAs (DMA instruction issue on the Sync
  engine costs ~600ns regardless of size).
- A short warmup matmul burst holds the PE busy so the HAM clock gate
  opens to 2.4 GHz before the real work arrives.
- Optional fp8 (E4M3, TRN max +-240) for the weight streams with
  per-output-row scales applied to the matmul results; small
  activations are pre-scaled by 16 (folded into the weight scales).
"""

import numpy as np
from contextlib import ExitStack

import ml_dtypes

import concourse.bass as bass
import concourse.mybir as mybir
import concourse.tile as tile
from concourse import bacc
from concourse.bass_utils import run_bass_kernel_spmd

H = 1024
V = 50257
L = 15          # MAX_LEN
P = 128
KH = H // P     # 8 k-tiles per H-vector
K2 = 2 * H // P # 16 k-tiles per 2H-vector
NCORES = 8
CHUNK = 512

AF = mybir.ActivationFunctionType
F32 = mybir.dt.float32
BF16 = mybir.dt.bfloat16
PAD_BIAS = -10000.0  # exp() underflows to exactly 0.0
XSCALE = 16.0        # pre-scale for small fp8 activations


def _dt(name):
    return {"float32": F32, "bfloat16": BF16,
            "float8e4": mybir.dt.float8e4}[name]


def _np_dt(name):
    return {"float32": np.float32, "bfloat16": ml_dtypes.bfloat16,
            "float8e4": ml_dtypes.float8_e4m3}[name]


def build_kernel(w_dtype="bfloat16", nch=13, num_cores=NCORES, out_dtype=None):
    """Build + compile the SPMD bass program. Returns compiled nc."""
    wdt = _dt(w_dtype)
    odt = _dt(out_dtype or w_dtype)
    fp32 = wdt == F32
    rdt = F32 if fp32 else BF16  # dtype of bias/scale rows
    vsh = nch * CHUNK  # padded vocab shard per core

    nc = bacc.Bacc(
        "TRN2",
        target_bir_lowering=False,
        debug=False,
        enable_asserts=True,
        num_devices=num_cores,
    )

    # ---- I/O ----
    emb_kt = nc.dram_tensor("emb_kt", [P, KH], F32, kind="ExternalInput").ap()
    h0_kt = nc.dram_tensor("h0_kt", [P, KH], F32, kind="ExternalInput").ap()
    h0_row = nc.dram_tensor("h0_row", [1, H], F32, kind="ExternalInput").ap()
    enc = nc.dram_tensor("enc", [L, H], F32, kind="ExternalInput").ap()
    attn_WT = nc.dram_tensor("attn_WT", [P, K2, L], F32, kind="ExternalInput").ap()
    attn_bT = nc.dram_tensor("attn_bT", [L, 1], F32, kind="ExternalInput").ap()
    attn_b = nc.dram_tensor("attn_b", [1, L], F32, kind="ExternalInput").ap()
    comb_WT = nc.dram_tensor("comb_WT", [2 * H, H], wdt, kind="ExternalInput").ap()
    w_ihT = nc.dram_tensor("w_ihT", [H, 3 * H], wdt, kind="ExternalInput").ap()
    w_hhT = nc.dram_tensor("w_hhT", [H, 3 * H], wdt, kind="ExternalInput").ap()
    b_comb_row = nc.dram_tensor("b_comb_row", [1, H], rdt, kind="ExternalInput").ap()
    b_ih_row = nc.dram_tensor("b_ih_row", [1, 3 * H], rdt, kind="ExternalInput").ap()
    b_hh_row = nc.dram_tensor("b_hh_row", [1, 3 * H], rdt, kind="ExternalInput").ap()
    ws_comb = nc.dram_tensor("ws_comb", [1, H], rdt, kind="ExternalInput").ap()
    ws_ih = nc.dram_tensor("ws_ih", [1, 3 * H], rdt, kind="ExternalInput").ap()
    ws_hh = nc.dram_tensor("ws_hh", [1, 3 * H], rdt, kind="ExternalInput").ap()
    outWT = nc.dram_tensor("outWT", [H, vsh], odt, kind="ExternalInput").ap()
    outb = nc.dram_tensor("outb", [1, vsh], rdt, kind="ExternalInput").ap()
    outs = nc.dram_tensor("outs", [1, vsh], rdt, kind="ExternalInput").ap()

    logits_s = nc.dram_tensor("logits_s", [1, vsh], F32, kind="ExternalOutput").ap()
    sumexp = nc.dram_tensor("sumexp", [1, 1], F32, kind="ExternalOutput").ap()
    h_new_kt = nc.dram_tensor("h_new_kt", [P, KH], F32, kind="ExternalOutput").ap()
    attn_w_out = nc.dram_tensor("attn_w", [1, L], F32, kind="ExternalOutput").ap()

    with tile.TileContext(nc) as tc:
        with ExitStack() as ctx:
            _body(ctx, tc, wdt, odt, nch, vsh, locals())

    nc.compile()
    return nc


def _body(ctx, tc, wdt, odt, nch, vsh, t):
    nc = tc.nc
    fp32 = wdt == F32
    w_scaled = wdt == mybir.dt.float8e4
    o_scaled = odt == mybir.dt.float8e4

    consts = ctx.enter_context(tc.tile_pool(name="consts", bufs=1))
    cpool = ctx.enter_context(tc.tile_pool(name="cpool", bufs=4))
    wpool = ctx.enter_context(tc.tile_pool(name="wpool", bufs=12))
    opool = ctx.enter_context(tc.tile_pool(name="opool", bufs=4 if fp32 else 8))
    pp_s1 = ctx.enter_context(
        tc.tile_pool(name="pp_s1", bufs=2, space=bass.MemorySpace.PSUM))
    pp_s2 = pp_s1
    pp_log = ctx.enter_context(
        tc.tile_pool(name="pp_log", bufs=6, space=bass.MemorySpace.PSUM))

    # ---- PE warmup: hold the systolic array busy so the HAM clock gate
    # opens before the real matmuls arrive (cold PE runs at 1.2 GHz).
    wu_w = consts.tile([P, P], F32)
    nc.vector.memset(wu_w[:], 0.0)
    wu_m = consts.tile([P, CHUNK], F32)
    nc.vector.memset(wu_m[:], 0.0)
    for i in range(10):
        wu_ps = pp_log.tile([1, CHUNK], F32, tag="lg", name=f"wu_{i}")
        nc.tensor.matmul(wu_ps[:], wu_w[:, 0:1], wu_m[:], start=True, stop=True)

    # ---- constant / small loads ----
    emb_t = consts.tile([P, KH], F32)
    nc.sync.dma_start(out=emb_t[:], in_=t["emb_kt"])
    h0_t = consts.tile([P, KH], F32)
    nc.sync.dma_start(out=h0_t[:], in_=t["h0_kt"])
    h0_row_t = consts.tile([1, H], F32)
    nc.sync.dma_start(out=h0_row_t[:], in_=t["h0_row"])
    enc_t = consts.tile([L, H], F32)
    nc.sync.dma_start(out=enc_t[:], in_=t["enc"])
    attnW_t = consts.tile([P, K2, L], F32)
    nc.sync.dma_start(out=attnW_t[:], in_=t["attn_WT"])
    attn_bT_t = consts.tile([L, 1], F32)
    nc.sync.dma_start(out=attn_bT_t[:], in_=t["attn_bT"])
    attn_b_t = consts.tile([1, L], F32)
    nc.sync.dma_start(out=attn_b_t[:], in_=t["attn_b"])
    rdt = F32 if fp32 else BF16
    b_comb_t = consts.tile([1, H], rdt)
    nc.sync.dma_start(out=b_comb_t[:], in_=t["b_comb_row"])
    b_ih_t = consts.tile([1, 3 * H], rdt)
    nc.sync.dma_start(out=b_ih_t[:], in_=t["b_ih_row"])
    b_hh_t = consts.tile([1, 3 * H], rdt)
    nc.sync.dma_start(out=b_hh_t[:], in_=t["b_hh_row"])
    outb_t = consts.tile([1, vsh], rdt)
    nc.sync.dma_start(out=outb_t[:], in_=t["outb"])
    if w_scaled:
        ws_comb_t = consts.tile([1, H], rdt)
        nc.sync.dma_start(out=ws_comb_t[:], in_=t["ws_comb"])
        ws_ih_t = consts.tile([1, 3 * H], rdt)
        nc.sync.dma_start(out=ws_ih_t[:], in_=t["ws_ih"])
        ws_hh_t = consts.tile([1, 3 * H], rdt)
        nc.sync.dma_start(out=ws_hh_t[:], in_=t["ws_hh"])
    if o_scaled:
        outs_t = consts.tile([1, vsh], rdt)
        nc.sync.dma_start(out=outs_t[:], in_=t["outs"])

    ones_t = consts.tile([1, P], F32)
    nc.vector.memset(ones_t[:], 1.0)
    one1 = consts.tile([1, 1], F32)
    nc.vector.memset(one1[:], 1.0)
    one1_w = one1
    if wdt != F32:
        one1_w = consts.tile([1, 1], wdt)
        nc.vector.memset(one1_w[:], 1.0)

    attn_in = consts.tile([P, K2], F32)
    nc.sync.dma_start(out=attn_in[:, 0:KH], in_=t["emb_kt"])
    nc.sync.dma_start(out=attn_in[:, KH:K2], in_=t["h0_kt"])
    comb_in = consts.tile([P, K2], F32)
    nc.sync.dma_start(out=comb_in[:, 0:KH], in_=t["emb_kt"])

    # ---- big weight loads: few large DMAs, engines round-robin ----
    cw_tiles = []  # 8 tiles of [128, 2, 1024] = comb_WT k-tile pairs
    for a in range(KH):
        cw = cpool.tile([P, 2, H], wdt, tag="cw", name=f"cw_{a}")
        src = t["comb_WT"][2 * P * a:2 * P * (a + 1), :].rearrange(
            "(a p) c -> p a c", p=P)
        nc.sync.dma_start(out=cw[:], in_=src)
        cw_tiles.append(cw)
    # ---- attention scores (both layouts) ----
    scores_ps = pp_s1.tile([1, L], F32, tag="s1")
    for k in range(K2):
        nc.tensor.matmul(scores_ps[:], attn_in[:, k:k + 1], attnW_t[:, k, :],
                         start=(k == 0), stop=(k == K2 - 1))
    scoresT_ps = pp_s2.tile([L, 1], F32, tag="s1", name="scoresT")
    for k in range(K2):
        nc.tensor.matmul(scoresT_ps[:], attnW_t[:, k, :], attn_in[:, k:k + 1],
                         start=(k == 0), stop=(k == K2 - 1))

    # attn_weights output = softmax(scores + b); no max-subtraction (|s| ~ 1)
    scores_sb = consts.tile([1, L], F32)
    nc.vector.tensor_add(scores_sb[:], scores_ps[:], attn_b_t[:])
    exp_sb = consts.tile([1, L], F32)
    se_sb = consts.tile([1, 1], F32)
    nc.scalar.activation(exp_sb[:], scores_sb[:], AF.Exp, accum_out=se_sb[:])
    inv_se = consts.tile([1, 1], F32)
    nc.vector.reciprocal(inv_se[:], se_sb[:])
    attn_w_sb = consts.tile([1, L], F32)
    nc.vector.tensor_scalar_mul(attn_w_sb[:], exp_sb[:], inv_se[:])
    nc.sync.dma_start(out=t["attn_w_out"], in_=attn_w_sb[:])

    # transposed exp(scores) on 15 partitions for the applied matmul
    expT_sb = consts.tile([L, 1], F32)
    nc.scalar.activation(expT_sb[:], scoresT_ps[:], AF.Exp, bias=attn_bT_t[:])

    # attn_applied^T (unnormalized): [128, 8] blocks = enc^T @ expT
    aa_ps = pp_s1.tile([P, KH], F32, tag="s1")
    for m in range(KH):
        nc.tensor.matmul(aa_ps[:, m:m + 1], enc_t[:, m * P:(m + 1) * P],
                         expT_sb[:], start=True, stop=True)

    # broadcast sum(exp) to 128 partitions: ones^T @ se
    bc_ps = pp_s2.tile([P, 1], F32, tag="s1", name="bc")
    nc.tensor.matmul(bc_ps[:], ones_t[:], se_sb[:], start=True, stop=True)
    inv_bc = consts.tile([P, 1], F32)
    nc.vector.reciprocal(inv_bc[:], bc_ps[:])
    nc.vector.tensor_scalar_mul(comb_in[:, KH:K2], aa_ps[:], inv_bc[:])

    comb_in_mm = comb_in
    if not fp32:
        comb_in_mm = consts.tile([P, K2], wdt)
        if w_scaled:  # pre-scale small activations into fp8's sweet range
            nc.vector.tensor_scalar_mul(comb_in_mm[:], comb_in[:], XSCALE)
        else:
            nc.vector.tensor_copy(comb_in_mm[:], comb_in[:])

    # ---- combine layer, row form: x_row = comb_in @ comb_WT + b ----
    x_row = consts.tile([1, H], F32)
    for c in range(H // CHUNK):
        csl = slice(c * CHUNK, (c + 1) * CHUNK)
        xr_ps = pp_log.tile([1, CHUNK], F32, tag="lg", name=f"xr_{c}")
        for k in range(K2):
            nc.tensor.matmul(xr_ps[:], comb_in_mm[:, k:k + 1],
                             cw_tiles[k // 2][:, k % 2, csl],
                             start=(k == 0), stop=(k == K2 - 1))
        if w_scaled:
            nc.vector.tensor_mul(x_row[:, csl], xr_ps[:], ws_comb_t[:, csl])
            nc.vector.tensor_add(x_row[:, csl], x_row[:, csl], b_comb_t[:, csl])
        else:
            nc.vector.tensor_add(x_row[:, csl], xr_ps[:], b_comb_t[:, csl])
    nc.vector.tensor_scalar_max(x_row[:], x_row[:], 0.0)
    x_mm_row = x_row
    if not fp32:
        x_mm_row = consts.tile([1, H], wdt)
        if w_scaled:
            nc.vector.tensor_scalar_mul(x_mm_row[:], x_row[:], XSCALE)
        else:
            nc.vector.tensor_copy(x_mm_row[:], x_row[:])

    # convert x row -> k-tile columns via K=1 matmuls (row_chunk.T @ [1])
    xT_ps = pp_s1.tile([P, KH], F32, tag="s1")
    for m in range(KH):
        nc.tensor.matmul(xT_ps[:, m:m + 1], x_mm_row[:, m * P:(m + 1) * P],
                         one1_w[:], start=True, stop=True)
    x_mm = consts.tile([P, KH], F32 if fp32 else wdt)
    nc.vector.tensor_copy(x_mm[:], xT_ps[:])

    h0_mm = h0_t
    if not fp32:
        h0_mm = consts.tile([P, KH], wdt)
        nc.vector.tensor_copy(h0_mm[:], h0_t[:])

    # ---- GRU gates, row form: gi = x @ w_ihT + b_ih ; gh = h0 @ w_hhT + b_hh
    # Each finished row chunk converts to [128, 24] psum columns right away
    # (K=1 matmuls) so the gate elementwise runs wide and h comes out in
    # k-tile layout directly.
    M3 = 3 * H // P
    HW3 = 3 * H // 2
    gi_row = consts.tile([1, 3 * H], F32)
    gh_row = consts.tile([1, 3 * H], F32)
    gi_cols = pp_s1.tile([P, M3], F32, tag="s1")
    gh_cols = pp_s2.tile([P, M3], F32, tag="s1", name="gh_cols")
    gate_srcs = ((t["w_ihT"], x_mm, gi_row, gi_cols, b_ih_t,
                  ws_ih_t if w_scaled else None),
                 (t["w_hhT"], h0_mm, gh_row, gh_cols, b_hh_t,
                  ws_hh_t if w_scaled else None))
    for gidx, (src, vec, row, cols, brow, ws_t) in enumerate(gate_srcs):
        for half in range(2):
            wts = []
            for k in range(KH):
                wt = wpool.tile([P, HW3], wdt, tag="wt",
                                name=f"wt_{gidx}_{half}_{k}")
                nc.sync.dma_start(
                    out=wt[:],
                    in_=src[k * P:(k + 1) * P, half * HW3:(half + 1) * HW3])
                wts.append(wt)
            for c in range(HW3 // CHUNK):
                gc = half * (HW3 // CHUNK) + c
                csl = slice(gc * CHUNK, (gc + 1) * CHUNK)
                lsl = slice(c * CHUNK, (c + 1) * CHUNK)
                gr_ps = pp_log.tile([1, CHUNK], F32, tag="lg",
                                    name=f"gr_{gidx}_{gc}")
                for k in range(KH):
                    nc.tensor.matmul(gr_ps[:], vec[:, k:k + 1], wts[k][:, lsl],
                                     start=(k == 0), stop=(k == KH - 1))
                if ws_t is not None:
                    nc.vector.tensor_mul(row[:, csl], gr_ps[:], ws_t[:, csl])
                    nc.vector.tensor_add(row[:, csl], row[:, csl], brow[:, csl])
                else:
                    nc.vector.tensor_add(row[:, csl], gr_ps[:], brow[:, csl])
                for m in range(CHUNK // P):
                    col = gc * (CHUNK // P) + m
                    nc.tensor.matmul(
                        cols[:, col:col + 1],
                        row[:, col * P:(col + 1) * P], one1[:],
                        start=True, stop=True)

    # out_WT loads issue after the gates weights so the bandwidth-saturated
    # DMA stream doesn't starve the (earlier-needed) GRU weight tiles;
    # 6.6MB still lands well before the logits matmuls start.
    ot_tiles = []
    for k in range(KH):
        ot = opool.tile([P, vsh], odt, tag="ot", name=f"ot_{k}")
        hw = vsh // 2
        nc.sync.dma_start(out=ot[:, 0:hw], in_=t["outWT"][k * P:(k + 1) * P, 0:hw])
        nc.sync.dma_start(out=ot[:, hw:vsh],
                          in_=t["outWT"][k * P:(k + 1) * P, hw:vsh])
        ot_tiles.append(ot)

    # ---- gates elementwise on [128, 8] k-tile columns ----
    gi_sb = consts.tile([P, M3], F32)
    nc.scalar.activation(gi_sb[:], gi_cols[:], AF.Copy)
    r_t = consts.tile([P, KH], F32)
    nc.vector.tensor_add(r_t[:], gi_sb[:, 0:KH], gh_cols[:, 0:KH])
    nc.scalar.activation(r_t[:], r_t[:], AF.Sigmoid)
    z_t = consts.tile([P, KH], F32)
    nc.vector.tensor_add(z_t[:], gi_sb[:, KH:2 * KH], gh_cols[:, KH:2 * KH])
    nc.scalar.activation(z_t[:], z_t[:], AF.Sigmoid)
    n_t = consts.tile([P, KH], F32)
    nc.vector.tensor_mul(n_t[:], r_t[:], gh_cols[:, 2 * KH:3 * KH])
    nc.vector.tensor_add(n_t[:], n_t[:], gi_sb[:, 2 * KH:3 * KH])
    nc.scalar.activation(n_t[:], n_t[:], AF.Tanh)
    # h_new = n + z * (h0 - n)
    d_t = consts.tile([P, KH], F32)
    nc.vector.tensor_sub(d_t[:], h0_t[:], n_t[:])
    h_t = consts.tile([P, KH], F32)
    nc.vector.tensor_mul(h_t[:], z_t[:], d_t[:])
    nc.vector.tensor_add(h_t[:], h_t[:], n_t[:])
    nc.sync.dma_start(out=t["h_new_kt"], in_=h_t[:])

    h_mm = h_t
    if odt != F32:
        h_mm = consts.tile([P, KH], odt)
        nc.vector.tensor_copy(h_mm[:], h_t[:])

    # ---- output projection: resident out_WT tiles, h stationary per pass ----
    lpool = ctx.enter_context(tc.tile_pool(name="lpool", bufs=3))
    se_slots = consts.tile([1, nch], F32)
    exp_tmp = consts.tile([1, CHUNK], F32)
    PASS = 5  # concurrent PSUM banks per pass
    for p0 in range(0, nch, PASS):
        cs = list(range(p0, min(p0 + PASS, nch)))
        lg_pss = {c: pp_log.tile([1, CHUNK], F32, tag="lg", name=f"lg_{c}")
                  for c in cs}
        for k in range(KH):
            for c in cs:
                nc.tensor.matmul(
                    lg_pss[c][:], h_mm[:, k:k + 1],
                    ot_tiles[k][:, c * CHUNK:(c + 1) * CHUNK],
                    start=(k == 0), stop=(k == KH - 1))
        for c in cs:
            csl = slice(c * CHUNK, (c + 1) * CHUNK)
            lg = lpool.tile([1, CHUNK], F32, tag="lgout", name=f"lgo_{c}")
            if o_scaled:
                nc.vector.tensor_mul(lg[:], lg_pss[c][:], outs_t[:, csl])
                nc.vector.tensor_add(lg[:], lg[:], outb_t[:, csl])
            else:
                nc.vector.tensor_add(lg[:], lg_pss[c][:], outb_t[:, csl])
            nc.scalar.activation(exp_tmp[:], lg[:], AF.Exp,
                                 accum_out=se_slots[:, c:c + 1])
            nc.sync.dma_start(out=t["logits_s"][:, csl], in_=lg[:])
    se_total = consts.tile([1, 1], F32)
    nc.vector.reduce_sum(se_total[:], se_slots[:], axis=mybir.AxisListType.X)
    nc.sync.dma_start(out=t["sumexp"], in_=se_total[:])


# ---------------------------------------------------------------------------
# host side
# ---------------------------------------------------------------------------

_CACHE = {}


def _get_nc(w_dtype, nch, out_dtype=None):
    key = (w_dtype, nch, out_dtype or w_dtype)
    if key not in _CACHE:
        _CACHE[key] = build_kernel(w_dtype, nch, out_dtype=out_dtype)
    return _CACHE[key]


def _kt(v):
    """[H*k] vector -> [128, k] partition-major tile layout."""
    v = np.asarray(v, np.float32)
    return np.ascontiguousarray(v.reshape(-1, P).T)


def _quant_rows(Wf, xscale):
    """fp8-quantize W [out, in] with per-out-row scales; returns (Wq, s_eff)
    where result_row = (x*xscale) @ Wq_row -> * s_eff recovers x @ W."""
    s = np.abs(Wf).max(axis=1) / 224.0
    s = np.maximum(s, 1e-30)
    Wq = np.clip(Wf / s[:, None], -240.0, 240.0)
    return Wq, (s / xscale).astype(np.float32)


def make_in_maps(inputs, w_dtype="bfloat16", nch=13, out_dtype=None):
    wnp = _np_dt(w_dtype)
    out_dtype = out_dtype or w_dtype
    onp = _np_dt(out_dtype)
    rnp = np.float32 if w_dtype == "float32" else ml_dtypes.bfloat16
    vsh = nch * CHUNK
    vpad = vsh * NCORES

    idx = int(np.asarray(inputs["input_seq"]).reshape(-1)[0])
    emb_row = np.asarray(inputs["emb_W"], np.float32)[idx]
    h0 = np.asarray(inputs["hidden"], np.float32).reshape(H)
    attn_W = np.asarray(inputs["attn_W"], np.float32)
    attn_b = np.asarray(inputs["attn_b"], np.float32)

    attn_WT = np.ascontiguousarray(
        attn_W.T.reshape(K2, P, L).transpose(1, 0, 2))  # [128, 16, 15]

    comb_W = np.asarray(inputs["comb_W"], np.float32)
    w_ih = np.asarray(inputs["w_ih"], np.float32)
    w_hh = np.asarray(inputs["w_hh"], np.float32)
    ws_comb = np.ones(H, np.float32)
    ws_ih = np.ones(3 * H, np.float32)
    ws_hh = np.ones(3 * H, np.float32)
    if w_dtype == "float8e4":
        comb_Wq, ws_comb = _quant_rows(comb_W, XSCALE)
        w_ihq, ws_ih = _quant_rows(w_ih, XSCALE)
        w_hhq, ws_hh = _quant_rows(w_hh, 1.0)  # h0 is not pre-scaled
        comb_WT = np.ascontiguousarray(comb_Wq.T.astype(wnp))
        w_ihT = np.ascontiguousarray(w_ihq.T.astype(wnp))
        w_hhT = np.ascontiguousarray(w_hhq.T.astype(wnp))
    else:
        comb_WT = np.ascontiguousarray(comb_W.T.astype(wnp))
        w_ihT = np.ascontiguousarray(w_ih.T.astype(wnp))
        w_hhT = np.ascontiguousarray(w_hh.T.astype(wnp))

    out_W = np.asarray(inputs["out_W"], np.float32)
    outs_pad = np.ones(vpad, np.float32)
    if out_dtype == "float8e4":
        out_Wq, s_eff = _quant_rows(out_W, 1.0)
        outs_pad[:V] = s_eff
        outWT_pad = np.zeros((H, vpad), onp)
        outWT_pad[:, :V] = out_Wq.T.astype(onp)
    else:
        outWT_pad = np.zeros((H, vpad), onp)
        outWT_pad[:, :V] = out_W.T.astype(onp)
    outb_pad = np.full(vpad, PAD_BIAS, np.float32)
    outb_pad[:V] = np.asarray(inputs["out_b"], np.float32)

    common = dict(
        emb_kt=_kt(emb_row),
        h0_kt=_kt(h0),
        h0_row=np.ascontiguousarray(h0.reshape(1, H)),
        enc=np.ascontiguousarray(np.asarray(inputs["encoder_outputs"], np.float32)),
        attn_WT=attn_WT,
        attn_bT=np.ascontiguousarray(attn_b.reshape(L, 1)),
        attn_b=np.ascontiguousarray(attn_b.reshape(1, L)),
        comb_WT=comb_WT,
        w_ihT=w_ihT,
        w_hhT=w_hhT,
        b_comb_row=np.asarray(inputs["comb_b"], np.float32).reshape(1, H).astype(rnp),
        b_ih_row=np.asarray(inputs["b_ih"], np.float32).reshape(1, 3 * H).astype(rnp),
        b_hh_row=np.asarray(inputs["b_hh"], np.float32).reshape(1, 3 * H).astype(rnp),
        ws_comb=ws_comb.reshape(1, H).astype(rnp),
        ws_ih=ws_ih.reshape(1, 3 * H).astype(rnp),
        ws_hh=ws_hh.reshape(1, 3 * H).astype(rnp),
    )
    in_maps = []
    for c in range(NCORES):
        m = dict(common)
        m["outWT"] = np.ascontiguousarray(outWT_pad[:, c * vsh:(c + 1) * vsh])
        m["outb"] = np.ascontiguousarray(
            outb_pad[c * vsh:(c + 1) * vsh].reshape(1, vsh).astype(rnp))
        m["outs"] = np.ascontiguousarray(
            outs_pad[c * vsh:(c + 1) * vsh].reshape(1, vsh).astype(rnp))
        in_maps.append(m)
    return in_maps


def assemble(results, nch=13):
    vsh = nch * CHUNK
    logits = np.concatenate(
        [results[c]["logits_s"].reshape(vsh) for c in range(NCORES)])[:V]
    total = np.float32(sum(float(np.asarray(results[c]["sumexp"]).reshape(-1)[0])
                           for c in range(NCORES)))
    out = (logits - np.float32(np.log(total))).reshape(1, V).astype(np.float32)
    h_new = np.ascontiguousarray(
        results[0]["h_new_kt"].T.reshape(1, 1, H).astype(np.float32))
    attn_w = np.ascontiguousarray(results[0]["attn_w"].reshape(1, L).astype(np.float32))
    return out, h_new, attn_w


def run(inputs, w_dtype="bfloat16", nch=13, trace=False, out_dtype=None):
    nc = _get_nc(w_dtype, nch, out_dtype)
    in_maps = make_in_maps(inputs, w_dtype, nch, out_dtype)
    res = run_bass_kernel_spmd(nc, in_maps, list(range(NCORES)), trace=trace)
    return assemble(res.results, nch), res


def kernel(**inputs):
    (out, h_new, attn_w), _ = run(inputs, w_dtype="bfloat16",
                                  out_dtype="float8e4")
    return out, h_new, attn_w


# revision 38
# speedup vs baseline: 1.0470x; 1.0470x over previous
"""AttnDecoderRNN step on 8 Trainium2 NeuronCores.

Sharding: out_W column-parallel (vocab) across 8 cores; tiny attention+GRU
replicated on every core; embedding row gathered on host (4KB of emb_W);
log_softmax normalizer combined on host from per-core sum(exp(logits)).

Device kernel notes:
- 1024-dim stationary vectors live as [128, 8] k-tile columns
  (partition p, col k <-> element 128k+p).
- comb/GRU matmuls run in row form (vector columns stationary, weight
  k-row chunks moving); rows convert back to k-tile columns via K=1
  matmuls against ones[1,1].
- Weight streams use few large DMAs (DMA instruction issue on the Sync
  engine costs ~600ns regardless of size).
- A short warmup matmul burst holds the PE busy so the HAM clock gate
  opens to 2.4 GHz before the real work arrives.
- Optional fp8 (E4M3, TRN max +-240) for the weight streams with
  per-output-row scales applied to the matmul results; small
  activations are pre-scaled by 16 (folded into the weight scales).
"""

import numpy as np
from contextlib import ExitStack

import ml_dtypes

import concourse.bass as bass
import concourse.mybir as mybir
import concourse.tile as tile
from concourse import bacc
from concourse.bass_utils import run_bass_kernel_spmd

H = 1024
V = 50257
L = 15          # MAX_LEN
P = 128
KH = H // P     # 8 k-tiles per H-vector
K2 = 2 * H // P # 16 k-tiles per 2H-vector
NCORES = 8
CHUNK = 512

AF = mybir.ActivationFunctionType
F32 = mybir.dt.float32
BF16 = mybir.dt.bfloat16
PAD_BIAS = -10000.0  # exp() underflows to exactly 0.0
XSCALE = 16.0        # pre-scale for small fp8 activations


def _dt(name):
    return {"float32": F32, "bfloat16": BF16,
            "float8e4": mybir.dt.float8e4}[name]


def _np_dt(name):
    return {"float32": np.float32, "bfloat16": ml_dtypes.bfloat16,
            "float8e4": ml_dtypes.float8_e4m3}[name]


def build_kernel(w_dtype="bfloat16", nch=13, num_cores=NCORES, out_dtype=None):
    """Build + compile the SPMD bass program. Returns compiled nc."""
    wdt = _dt(w_dtype)
    odt = _dt(out_dtype or w_dtype)
    fp32 = wdt == F32
    rdt = F32 if fp32 else BF16  # dtype of bias/scale rows
    vsh = nch * CHUNK  # padded vocab shard per core

    nc = bacc.Bacc(
        "TRN2",
        target_bir_lowering=False,
        debug=False,
        enable_asserts=True,
        num_devices=num_cores,
    )

    # ---- I/O ----
    emb_kt = nc.dram_tensor("emb_kt", [P, KH], F32, kind="ExternalInput").ap()
    h0_kt = nc.dram_tensor("h0_kt", [P, KH], F32, kind="ExternalInput").ap()
    h0_row = nc.dram_tensor("h0_row", [1, H], F32, kind="ExternalInput").ap()
    enc = nc.dram_tensor("enc", [L, H], F32, kind="ExternalInput").ap()
    attn_WT = nc.dram_tensor("attn_WT", [P, K2, L], F32, kind="ExternalInput").ap()
    attn_bT = nc.dram_tensor("attn_bT", [L, 1], F32, kind="ExternalInput").ap()
    attn_b = nc.dram_tensor("attn_b", [1, L], F32, kind="ExternalInput").ap()
    comb_WT = nc.dram_tensor("comb_WT", [2 * H, H], wdt, kind="ExternalInput").ap()
    w_ihT = nc.dram_tensor("w_ihT", [H, 3 * H], wdt, kind="ExternalInput").ap()
    w_hhT = nc.dram_tensor("w_hhT", [H, 3 * H], wdt, kind="ExternalInput").ap()
    b_comb_row = nc.dram_tensor("b_comb_row", [1, H], rdt, kind="ExternalInput").ap()
    b_ih_row = nc.dram_tensor("b_ih_row", [1, 3 * H], rdt, kind="ExternalInput").ap()
    b_hh_row = nc.dram_tensor("b_hh_row", [1, 3 * H], rdt, kind="ExternalInput").ap()
    ws_comb = nc.dram_tensor("ws_comb", [1, H], rdt, kind="ExternalInput").ap()
    ws_ih = nc.dram_tensor("ws_ih", [1, 3 * H], rdt, kind="ExternalInput").ap()
    ws_hh = nc.dram_tensor("ws_hh", [1, 3 * H], rdt, kind="ExternalInput").ap()
    outWT = nc.dram_tensor("outWT", [H, vsh], odt, kind="ExternalInput").ap()
    outb = nc.dram_tensor("outb", [1, vsh], rdt, kind="ExternalInput").ap()
    outs = nc.dram_tensor("outs", [1, vsh], rdt, kind="ExternalInput").ap()

    logits_s = nc.dram_tensor("logits_s", [1, vsh], F32, kind="ExternalOutput").ap()
    sumexp = nc.dram_tensor("sumexp", [1, 1], F32, kind="ExternalOutput").ap()
    h_new_kt = nc.dram_tensor("h_new_kt", [P, KH], F32, kind="ExternalOutput").ap()
    attn_w_out = nc.dram_tensor("attn_w", [1, L], F32, kind="ExternalOutput").ap()

    with tile.TileContext(nc) as tc:
        with ExitStack() as ctx:
            _body(ctx, tc, wdt, odt, nch, vsh, locals())

    nc.compile()
    return nc


def _body(ctx, tc, wdt, odt, nch, vsh, t):
    nc = tc.nc
    fp32 = wdt == F32
    w_scaled = wdt == mybir.dt.float8e4
    o_scaled = odt == mybir.dt.float8e4

    consts = ctx.enter_context(tc.tile_pool(name="consts", bufs=1))
    cpool = ctx.enter_context(tc.tile_pool(name="cpool", bufs=4))
    wpool = ctx.enter_context(tc.tile_pool(name="wpool", bufs=12))
    opool = ctx.enter_context(tc.tile_pool(name="opool", bufs=4 if fp32 else 8))
    pp_s1 = ctx.enter_context(
        tc.tile_pool(name="pp_s1", bufs=2, space=bass.MemorySpace.PSUM))
    pp_s2 = ctx.enter_context(
        tc.tile_pool(name="pp_s2", bufs=1, space=bass.MemorySpace.PSUM))
    pp_log = ctx.enter_context(
        tc.tile_pool(name="pp_log", bufs=5, space=bass.MemorySpace.PSUM))

    # ---- PE warmup: hold the systolic array busy so the HAM clock gate
    # opens before the real matmuls arrive (cold PE runs at 1.2 GHz).
    wu_w = consts.tile([P, P], F32)
    nc.gpsimd.memset(wu_w[:], 0.0)
    wu_m = consts.tile([P, CHUNK], F32)
    nc.gpsimd.memset(wu_m[:], 0.0)
    for i in range(10):
        wu_ps = pp_log.tile([1, CHUNK], F32, tag="lg", name=f"wu_{i}")
        nc.tensor.matmul(wu_ps[:], wu_w[:, 0:1], wu_m[:], start=True, stop=True)

    # ---- constant / small loads ----
    emb_t = consts.tile([P, KH], F32)
    nc.sync.dma_start(out=emb_t[:], in_=t["emb_kt"])
    h0_t = consts.tile([P, KH], F32)
    nc.sync.dma_start(out=h0_t[:], in_=t["h0_kt"])
    h0_row_t = consts.tile([1, H], F32)
    nc.sync.dma_start(out=h0_row_t[:], in_=t["h0_row"])
    enc_t = consts.tile([L, H], F32)
    nc.sync.dma_start(out=enc_t[:], in_=t["enc"])
    attnW_t = consts.tile([P, K2, L], F32)
    nc.sync.dma_start(out=attnW_t[:], in_=t["attn_WT"])
    attn_bT_t = consts.tile([L, 1], F32)
    nc.sync.dma_start(out=attn_bT_t[:], in_=t["attn_bT"])
    attn_b_t = consts.tile([1, L], F32)
    nc.sync.dma_start(out=attn_b_t[:], in_=t["attn_b"])
    rdt = F32 if fp32 else BF16
    b_comb_t = consts.tile([1, H], rdt)
    nc.sync.dma_start(out=b_comb_t[:], in_=t["b_comb_row"])
    b_ih_t = consts.tile([1, 3 * H], rdt)
    nc.sync.dma_start(out=b_ih_t[:], in_=t["b_ih_row"])
    b_hh_t = consts.tile([1, 3 * H], rdt)
    nc.sync.dma_start(out=b_hh_t[:], in_=t["b_hh_row"])
    outb_t = consts.tile([1, vsh], rdt)
    nc.sync.dma_start(out=outb_t[:], in_=t["outb"])
    if w_scaled:
        ws_comb_t = consts.tile([1, H], rdt)
        nc.sync.dma_start(out=ws_comb_t[:], in_=t["ws_comb"])
        ws_ih_t = consts.tile([1, 3 * H], rdt)
        nc.sync.dma_start(out=ws_ih_t[:], in_=t["ws_ih"])
        ws_hh_t = consts.tile([1, 3 * H], rdt)
        nc.sync.dma_start(out=ws_hh_t[:], in_=t["ws_hh"])
    if o_scaled:
        outs_t = consts.tile([1, vsh], rdt)
        nc.sync.dma_start(out=outs_t[:], in_=t["outs"])

    ones_t = consts.tile([1, P], F32)
    nc.vector.memset(ones_t[:], 1.0)
    one1 = consts.tile([1, 1], F32)
    nc.vector.memset(one1[:], 1.0)
    one1_w = one1
    if wdt != F32:
        one1_w = consts.tile([1, 1], wdt)
        nc.vector.memset(one1_w[:], 1.0)

    attn_in = consts.tile([P, K2], F32)
    nc.sync.dma_start(out=attn_in[:, 0:KH], in_=t["emb_kt"])
    nc.sync.dma_start(out=attn_in[:, KH:K2], in_=t["h0_kt"])
    comb_in = consts.tile([P, K2], F32)
    nc.sync.dma_start(out=comb_in[:, 0:KH], in_=t["emb_kt"])

    # ---- big weight loads: few large DMAs, engines round-robin ----
    cw_tiles = []  # 8 tiles of [128, 2, 1024] = comb_WT k-tile pairs
    for a in range(KH):
        cw = cpool.tile([P, 2, H], wdt, tag="cw", name=f"cw_{a}")
        src = t["comb_WT"][2 * P * a:2 * P * (a + 1), :].rearrange(
            "(a p) c -> p a c", p=P)
        nc.sync.dma_start(out=cw[:], in_=src)
        cw_tiles.append(cw)
    # ---- attention scores (both layouts) ----
    scores_ps = pp_s1.tile([1, L], F32, tag="s1")
    for k in range(K2):
        nc.tensor.matmul(scores_ps[:], attn_in[:, k:k + 1], attnW_t[:, k, :],
                         start=(k == 0), stop=(k == K2 - 1))
    scoresT_ps = pp_s2.tile([L, 1], F32, tag="s2")
    for k in range(K2):
        nc.tensor.matmul(scoresT_ps[:], attnW_t[:, k, :], attn_in[:, k:k + 1],
                         start=(k == 0), stop=(k == K2 - 1))

    # attn_weights output = softmax(scores + b); no max-subtraction (|s| ~ 1)
    scores_sb = consts.tile([1, L], F32)
    nc.vector.tensor_add(scores_sb[:], scores_ps[:], attn_b_t[:])
    exp_sb = consts.tile([1, L], F32)
    se_sb = consts.tile([1, 1], F32)
    nc.scalar.activation(exp_sb[:], scores_sb[:], AF.Exp, accum_out=se_sb[:])
    inv_se = consts.tile([1, 1], F32)
    nc.vector.reciprocal(inv_se[:], se_sb[:])
    attn_w_sb = consts.tile([1, L], F32)
    nc.vector.tensor_scalar_mul(attn_w_sb[:], exp_sb[:], inv_se[:])
    nc.sync.dma_start(out=t["attn_w_out"], in_=attn_w_sb[:])

    # transposed exp(scores) on 15 partitions for the applied matmul
    expT_sb = consts.tile([L, 1], F32)
    nc.scalar.activation(expT_sb[:], scoresT_ps[:], AF.Exp, bias=attn_bT_t[:])

    # attn_applied^T (unnormalized): [128, 8] blocks = enc^T @ expT
    aa_ps = pp_s1.tile([P, KH], F32, tag="s1")
    for m in range(KH):
        nc.tensor.matmul(aa_ps[:, m:m + 1], enc_t[:, m * P:(m + 1) * P],
                         expT_sb[:], start=True, stop=True)

    # broadcast sum(exp) to 128 partitions: ones^T @ se
    bc_ps = pp_s2.tile([P, 1], F32, tag="s2")
    nc.tensor.matmul(bc_ps[:], ones_t[:], se_sb[:], start=True, stop=True)
    inv_bc = consts.tile([P, 1], F32)
    nc.vector.reciprocal(inv_bc[:], bc_ps[:])
    nc.vector.tensor_scalar_mul(comb_in[:, KH:K2], aa_ps[:], inv_bc[:])

    comb_in_mm = comb_in
    if not fp32:
        comb_in_mm = consts.tile([P, K2], wdt)
        if w_scaled:  # pre-scale small activations into fp8's sweet range
            nc.vector.tensor_scalar_mul(comb_in_mm[:], comb_in[:], XSCALE)
        else:
            nc.vector.tensor_copy(comb_in_mm[:], comb_in[:])

    # ---- combine layer, row form: x_row = comb_in @ comb_WT + b ----
    x_row = consts.tile([1, H], F32)
    for c in range(H // CHUNK):
        csl = slice(c * CHUNK, (c + 1) * CHUNK)
        xr_ps = pp_log.tile([1, CHUNK], F32, tag="lg", name=f"xr_{c}")
        for k in range(K2):
            nc.tensor.matmul(xr_ps[:], comb_in_mm[:, k:k + 1],
                             cw_tiles[k // 2][:, k % 2, csl],
                             start=(k == 0), stop=(k == K2 - 1))
        if w_scaled:
            nc.vector.tensor_mul(x_row[:, csl], xr_ps[:], ws_comb_t[:, csl])
            nc.vector.tensor_add(x_row[:, csl], x_row[:, csl], b_comb_t[:, csl])
        else:
            nc.vector.tensor_add(x_row[:, csl], xr_ps[:], b_comb_t[:, csl])
    nc.vector.tensor_scalar_max(x_row[:], x_row[:], 0.0)
    x_mm_row = x_row
    if not fp32:
        x_mm_row = consts.tile([1, H], wdt)
        if w_scaled:
            nc.vector.tensor_scalar_mul(x_mm_row[:], x_row[:], XSCALE)
        else:
            nc.vector.tensor_copy(x_mm_row[:], x_row[:])

    # convert x row -> k-tile columns via K=1 matmuls (row_chunk.T @ [1])
    xT_ps = pp_s1.tile([P, KH], F32, tag="s1")
    for m in range(KH):
        nc.tensor.matmul(xT_ps[:, m:m + 1], x_mm_row[:, m * P:(m + 1) * P],
                         one1_w[:], start=True, stop=True)
    x_mm = consts.tile([P, KH], F32 if fp32 else wdt)
    nc.vector.tensor_copy(x_mm[:], xT_ps[:])

    h0_mm = h0_t
    if not fp32:
        h0_mm = consts.tile([P, KH], wdt)
        nc.vector.tensor_copy(h0_mm[:], h0_t[:])

    # ---- GRU gates, row form: gi = x @ w_ihT + b_ih ; gh = h0 @ w_hhT + b_hh
    # Each finished row chunk converts to [128, 24] psum columns right away
    # (K=1 matmuls) so the gate elementwise runs wide and h comes out in
    # k-tile layout directly.
    M3 = 3 * H // P
    HW3 = 3 * H // 2
    gi_row = consts.tile([1, 3 * H], F32)
    gh_row = consts.tile([1, 3 * H], F32)
    gi_cols = pp_s1.tile([P, M3], F32, tag="s1")
    gh_cols = pp_s2.tile([P, M3], F32, tag="s2")
    gate_srcs = ((t["w_ihT"], x_mm, gi_row, gi_cols, b_ih_t,
                  ws_ih_t if w_scaled else None),
                 (t["w_hhT"], h0_mm, gh_row, gh_cols, b_hh_t,
                  ws_hh_t if w_scaled else None))
    for gidx, (src, vec, row, cols, brow, ws_t) in enumerate(gate_srcs):
        for half in range(2):
            wts = []
            for k in range(KH):
                wt = wpool.tile([P, HW3], wdt, tag="wt",
                                name=f"wt_{gidx}_{half}_{k}")
                nc.sync.dma_start(
                    out=wt[:],
                    in_=src[k * P:(k + 1) * P, half * HW3:(half + 1) * HW3])
                wts.append(wt)
            for c in range(HW3 // CHUNK):
                gc = half * (HW3 // CHUNK) + c
                csl = slice(gc * CHUNK, (gc + 1) * CHUNK)
                lsl = slice(c * CHUNK, (c + 1) * CHUNK)
                gr_ps = pp_log.tile([1, CHUNK], F32, tag="lg",
                                    name=f"gr_{gidx}_{gc}")
                for k in range(KH):
                    nc.tensor.matmul(gr_ps[:], vec[:, k:k + 1], wts[k][:, lsl],
                                     start=(k == 0), stop=(k == KH - 1))
                if ws_t is not None:
                    nc.vector.tensor_mul(row[:, csl], gr_ps[:], ws_t[:, csl])
                    nc.vector.tensor_add(row[:, csl], row[:, csl], brow[:, csl])
                else:
                    nc.vector.tensor_add(row[:, csl], gr_ps[:], brow[:, csl])
        # row->column conversions AFTER the full matmul stream: emitting them
        # per-chunk head-of-line blocks the PE queue on each chunk's DVE
        # bias-add (~1us x 14 chunks) and re-colds the HAM clock
        for col in range(M3):
            nc.tensor.matmul(cols[:, col:col + 1],
                             row[:, col * P:(col + 1) * P], one1[:],
                             start=True, stop=True)

    # out_WT loads issue after the gates weights so the bandwidth-saturated
    # DMA stream doesn't starve the (earlier-needed) GRU weight tiles;
    # 6.6MB still lands well before the logits matmuls start.
    ot_tiles = []
    for k in range(KH):
        ot = opool.tile([P, vsh], odt, tag="ot", name=f"ot_{k}")
        hw = vsh // 2
        nc.sync.dma_start(out=ot[:, 0:hw], in_=t["outWT"][k * P:(k + 1) * P, 0:hw])
        nc.sync.dma_start(out=ot[:, hw:vsh],
                          in_=t["outWT"][k * P:(k + 1) * P, hw:vsh])
        ot_tiles.append(ot)

    # ---- gates elementwise on [128, 8] k-tile columns ----
    gi_sb = consts.tile([P, M3], F32)
    nc.scalar.activation(gi_sb[:], gi_cols[:], AF.Copy)
    r_t = consts.tile([P, KH], F32)
    nc.vector.tensor_add(r_t[:], gi_sb[:, 0:KH], gh_cols[:, 0:KH])
    nc.scalar.activation(r_t[:], r_t[:], AF.Sigmoid)
    z_t = consts.tile([P, KH], F32)
    nc.vector.tensor_add(z_t[:], gi_sb[:, KH:2 * KH], gh_cols[:, KH:2 * KH])
    nc.scalar.activation(z_t[:], z_t[:], AF.Sigmoid)
    n_t = consts.tile([P, KH], F32)
    nc.vector.tensor_mul(n_t[:], r_t[:], gh_cols[:, 2 * KH:3 * KH])
    nc.vector.tensor_add(n_t[:], n_t[:], gi_sb[:, 2 * KH:3 * KH])
    nc.scalar.activation(n_t[:], n_t[:], AF.Tanh)
    # h_new = n + z * (h0 - n)
    d_t = consts.tile([P, KH], F32)
    nc.vector.tensor_sub(d_t[:], h0_t[:], n_t[:])
    h_t = consts.tile([P, KH], F32)
    nc.vector.tensor_mul(h_t[:], z_t[:], d_t[:])
    nc.vector.tensor_add(h_t[:], h_t[:], n_t[:])
    nc.sync.dma_start(out=t["h_new_kt"], in_=h_t[:])

    h_mm = h_t
    if odt != F32:
        h_mm = consts.tile([P, KH], odt)
        nc.vector.tensor_copy(h_mm[:], h_t[:])

    # ---- output projection: resident out_WT tiles, h stationary per pass ----
    lpool = ctx.enter_context(tc.tile_pool(name="lpool", bufs=3))
    se_slots = consts.tile([1, nch], F32)
    exp_tmp = consts.tile([1, CHUNK], F32)
    PASS = 5  # concurrent PSUM banks per pass
    for p0 in range(0, nch, PASS):
        cs = list(range(p0, min(p0 + PASS, nch)))
        lg_pss = {c: pp_log.tile([1, CHUNK], F32, tag="lg", name=f"lg_{c}")
                  for c in cs}
        for k in range(KH):
            for c in cs:
                nc.tensor.matmul(
                    lg_pss[c][:], h_mm[:, k:k + 1],
                    ot_tiles[k][:, c * CHUNK:(c + 1) * CHUNK],
                    start=(k == 0), stop=(k == KH - 1))
        for c in cs:
            csl = slice(c * CHUNK, (c + 1) * CHUNK)
            lg = lpool.tile([1, CHUNK], F32, tag="lgout", name=f"lgo_{c}")
            if o_scaled:
                nc.vector.tensor_mul(lg[:], lg_pss[c][:], outs_t[:, csl])
                nc.vector.tensor_add(lg[:], lg[:], outb_t[:, csl])
            else:
                nc.vector.tensor_add(lg[:], lg_pss[c][:], outb_t[:, csl])
            nc.scalar.activation(exp_tmp[:], lg[:], AF.Exp,
                                 accum_out=se_slots[:, c:c + 1])
            nc.sync.dma_start(out=t["logits_s"][:, csl], in_=lg[:])
    se_total = consts.tile([1, 1], F32)
    nc.vector.reduce_sum(se_total[:], se_slots[:], axis=mybir.AxisListType.X)
    nc.sync.dma_start(out=t["sumexp"], in_=se_total[:])


# ---------------------------------------------------------------------------
# host side
# ---------------------------------------------------------------------------

_CACHE = {}


def _get_nc(w_dtype, nch, out_dtype=None):
    key = (w_dtype, nch, out_dtype or w_dtype)
    if key not in _CACHE:
        _CACHE[key] = build_kernel(w_dtype, nch, out_dtype=out_dtype)
    return _CACHE[key]


def _kt(v):
    """[H*k] vector -> [128, k] partition-major tile layout."""
    v = np.asarray(v, np.float32)
    return np.ascontiguousarray(v.reshape(-1, P).T)


def _quant_rows(Wf, xscale):
    """fp8-quantize W [out, in] with per-out-row scales; returns (Wq, s_eff)
    where result_row = (x*xscale) @ Wq_row -> * s_eff recovers x @ W."""
    s = np.abs(Wf).max(axis=1) / 224.0
    s = np.maximum(s, 1e-30)
    Wq = np.clip(Wf / s[:, None], -240.0, 240.0)
    return Wq, (s / xscale).astype(np.float32)


def make_in_maps(inputs, w_dtype="bfloat16", nch=13, out_dtype=None):
    wnp = _np_dt(w_dtype)
    out_dtype = out_dtype or w_dtype
    onp = _np_dt(out_dtype)
    rnp = np.float32 if w_dtype == "float32" else ml_dtypes.bfloat16
    vsh = nch * CHUNK
    vpad = vsh * NCORES

    idx = int(np.asarray(inputs["input_seq"]).reshape(-1)[0])
    emb_row = np.asarray(inputs["emb_W"], np.float32)[idx]
    h0 = np.asarray(inputs["hidden"], np.float32).reshape(H)
    attn_W = np.asarray(inputs["attn_W"], np.float32)
    attn_b = np.asarray(inputs["attn_b"], np.float32)

    attn_WT = np.ascontiguousarray(
        attn_W.T.reshape(K2, P, L).transpose(1, 0, 2))  # [128, 16, 15]

    comb_W = np.asarray(inputs["comb_W"], np.float32)
    w_ih = np.asarray(inputs["w_ih"], np.float32)
    w_hh = np.asarray(inputs["w_hh"], np.float32)
    ws_comb = np.ones(H, np.float32)
    ws_ih = np.ones(3 * H, np.float32)
    ws_hh = np.ones(3 * H, np.float32)
    if w_dtype == "float8e4":
        comb_Wq, ws_comb = _quant_rows(comb_W, XSCALE)
        w_ihq, ws_ih = _quant_rows(w_ih, XSCALE)
        w_hhq, ws_hh = _quant_rows(w_hh, 1.0)  # h0 is not pre-scaled
        comb_WT = np.ascontiguousarray(comb_Wq.T.astype(wnp))
        w_ihT = np.ascontiguousarray(w_ihq.T.astype(wnp))
        w_hhT = np.ascontiguousarray(w_hhq.T.astype(wnp))
    else:
        comb_WT = np.ascontiguousarray(comb_W.T.astype(wnp))
        w_ihT = np.ascontiguousarray(w_ih.T.astype(wnp))
        w_hhT = np.ascontiguousarray(w_hh.T.astype(wnp))

    out_W = np.asarray(inputs["out_W"], np.float32)
    outs_pad = np.ones(vpad, np.float32)
    if out_dtype == "float8e4":
        out_Wq, s_eff = _quant_rows(out_W, 1.0)
        outs_pad[:V] = s_eff
        outWT_pad = np.zeros((H, vpad), onp)
        outWT_pad[:, :V] = out_Wq.T.astype(onp)
    else:
        outWT_pad = np.zeros((H, vpad), onp)
        outWT_pad[:, :V] = out_W.T.astype(onp)
    outb_pad = np.full(vpad, PAD_BIAS, np.float32)
    outb_pad[:V] = np.asarray(inputs["out_b"], np.float32)

    common = dict(
        emb_kt=_kt(emb_row),
        h0_kt=_kt(h0),
        h0_row=np.ascontiguousarray(h0.reshape(1, H)),
        enc=np.ascontiguousarray(np.asarray(inputs["encoder_outputs"], np.float32)),
        attn_WT=attn_WT,
        attn_bT=np.ascontiguousarray(attn_b.reshape(L, 1)),
        attn_b=np.ascontiguousarray(attn_b.reshape(1, L)),
        comb_WT=comb_WT,
        w_ihT=w_ihT,
        w_hhT=w_hhT,
        b_comb_row=np.asarray(inputs["comb_b"], np.float32).reshape(1, H).astype(rnp),
        b_ih_row=np.asarray(inputs["b_ih"], np.float32).reshape(1, 3 * H).astype(rnp),
        b_hh_row=np.asarray(inputs["b_hh"], np.float32).reshape(1, 3 * H).astype(rnp),
        ws_comb=ws_comb.reshape(1, H).astype(rnp),
        ws_ih=ws_ih.reshape(1, 3 * H).astype(rnp),
        ws_hh=ws_hh.reshape(1, 3 * H).astype(rnp),
    )
    in_maps = []
    for c in range(NCORES):
        m = dict(common)
        m["outWT"] = np.ascontiguousarray(outWT_pad[:, c * vsh:(c + 1) * vsh])
        m["outb"] = np.ascontiguousarray(
            outb_pad[c * vsh:(c + 1) * vsh].reshape(1, vsh).astype(rnp))
        m["outs"] = np.ascontiguousarray(
            outs_pad[c * vsh:(c + 1) * vsh].reshape(1, vsh).astype(rnp))
        in_maps.append(m)
    return in_maps


def assemble(results, nch=13):
    vsh = nch * CHUNK
    logits = np.concatenate(
        [results[c]["logits_s"].reshape(vsh) for c in range(NCORES)])[:V]
    total = np.float32(sum(float(np.asarray(results[c]["sumexp"]).reshape(-1)[0])
                           for c in range(NCORES)))
    out = (logits - np.float32(np.log(total))).reshape(1, V).astype(np.float32)
    h_new = np.ascontiguousarray(
        results[0]["h_new_kt"].T.reshape(1, 1, H).astype(np.float32))
    attn_w = np.ascontiguousarray(results[0]["attn_w"].reshape(1, L).astype(np.float32))
    return out, h_new, attn_w


def run(inputs, w_dtype="bfloat16", nch=13, trace=False, out_dtype=None):
    nc = _get_nc(w_dtype, nch, out_dtype)
    in_maps = make_in_maps(inputs, w_dtype, nch, out_dtype)
    res = run_bass_kernel_spmd(nc, in_maps, list(range(NCORES)), trace=trace)
    return assemble(res.results, nch), res


def kernel(**inputs):
    (out, h_new, attn_w), _ = run(inputs, w_dtype="bfloat16",
                                  out_dtype="float8e4")
    return out, h_new, attn_w
